# revision 18
# baseline (speedup 1.0000x reference)
"""Channel-attention (nn_ChannelAttentionModule) Trainium2 kernel.

Math (per batch b):
    X = x[b]  [C, N]  with C=512, N=64*64=4096
    q = Wq X + bq ; k = Wk X + bk ; v = Wv X + bv
    L = q k^T                       [C, C]
    out = softmax(L, -1) v + X      [C, N]

Key restructure: L = Wq G Wk^T + bq (Wk S + N bk)^T_outer + (Wq S) bk^T_outer
  where G = X X^T (Gram, symmetric) and S = X 1 (row sums).
This moves all precision-critical work into G (computed with a bf16
hi/lo split: G = Xh Xh^T + M + M^T, M = Xh Xl^T) plus two tiny fp32
512^3 matmuls, leaving the v-path in fast fp16.

Sharding: pure data-parallel, one batch per NeuronCore (B=8, 8 cores).
"""

import numpy as np
import ml_dtypes

import concourse.bass as bass
import concourse.mybir as mybir
import concourse.tile as tile
from concourse import bacc
from concourse.bass_utils import run_bass_kernel_spmd

F32 = mybir.dt.float32
BF16 = mybir.dt.bfloat16
F16 = mybir.dt.float16
F32R = mybir.dt.float32r
AX = mybir.AxisListType.X
EXP = mybir.ActivationFunctionType.Exp

B = 8
USE_F32R_G = True  # single f32r Gram pass instead of bf16 hi/lo split
C = 512
HW = 64 * 64  # N spatial
P = 128
CH = C // P  # 4 chunks of channels
NCH = HW // P  # 32 chunks of spatial (G pass)
NT = HW // 512  # 8 tiles of spatial (v / out pass)


def _body(tc, nc, io):
    x16 = io["x16"]
    if USE_F32R_G:
        xtr = io["xtr"]
    else:
        xth, xtl = io["xth"], io["xtl"]
    wqt, wkt, wvt = io["wqt"], io["wkt"], io["wvt"]
    bqr, bkr, nbkr, bvc = io["bqr"], io["bkr"], io["nbkr"], io["bvc"]
    id32, id16, out = io["id32"], io["id16"], io["out"]

    ps = tc.alloc_tile_pool(name="ps", bufs=1, space="PSUM")
    sb = tc.alloc_tile_pool(name="sb", bufs=1)
    st = tc.alloc_tile_pool(name="st", bufs=4)
    so = tc.alloc_tile_pool(name="so", bufs=2)

    # ---- G pass ----
    g_ps = [ps.tile([P, C], F32, name=f"gps{i}", tag=f"pa{i}") for i in range(CH)]
    if USE_F32R_G:
        # single-pass Gram in f32r (~11-bit input rounding), upper triangle
        # only; row 3 starts at block 2 so the f32r free dim stays >= 256.
        USTART = [0, 128, 256, 256]
        for n in range(NCH):
            ar = st.tile([P, C], F32R, name="ar", tag="ar")
            nc.sync.dma_start(ar, xtr[n * P : (n + 1) * P, :])
            first, last = n == 0, n == NCH - 1
            for c in range(CH):
                u = USTART[c]
                nc.tensor.matmul(
                    g_ps[c][:, u:], lhsT=ar[:, c * P : (c + 1) * P], rhs=ar[:, u:],
                    start=first, stop=last,
                )
    else:
        # bf16 hi/lo split: G = Xh Xh^T + M + M^T, M = Xh Xl^T
        m_ps = [ps.tile([P, C], F32, name=f"mps{i}", tag=f"pb{i}") for i in range(CH)]
        for n in range(NCH):
            ah = st.tile([P, C], BF16, name="ah", tag="ah")
            nc.sync.dma_start(ah, xth[n * P : (n + 1) * P, :])
            al = st.tile([P, C], BF16, name="al", tag="al")
            nc.sync.dma_start(al, xtl[n * P : (n + 1) * P, :])
            first, last = n == 0, n == NCH - 1
            for c in range(CH):
                lhs = ah[:, c * P : (c + 1) * P]
                nc.tensor.matmul(g_ps[c], lhsT=lhs, rhs=ah, start=first, stop=last)
                nc.tensor.matmul(m_ps[c], lhsT=lhs, rhs=al, start=first, stop=last)

    # ---- fp16 x in (v-path operand); S row sums from it on idle DVE ----
    x16_sb = [sb.tile([P, HW], F16, name=f"x16_{i}", tag=f"x16_{i}") for i in range(CH)]
    for i in range(CH):
        for sl in range(2):
            nc.sync.dma_start(
                x16_sb[i][:, sl * 2048 : (sl + 1) * 2048],
                x16[i * P : (i + 1) * P, sl * 2048 : (sl + 1) * 2048],
            )
    s_col = [sb.tile([P, 1], F32, name=f"s{i}", tag=f"s{i}") for i in range(CH)]
    for i in range(CH):
        nc.vector.reduce_sum(s_col[i], x16_sb[i], axis=AX)

    # ---- weights/consts in (needed ~60us in) ----
    wqt_sb = [sb.tile([P, C], F32, name=f"wqt{i}", tag=f"wqt{i}") for i in range(CH)]
    wkt_sb = [sb.tile([P, C], F32, name=f"wkt{i}", tag=f"wkt{i}") for i in range(CH)]
    wvt_sb = [sb.tile([P, C], F16, name=f"wvt{i}", tag=f"wvt{i}") for i in range(CH)]
    for i in range(CH):
        nc.sync.dma_start(wqt_sb[i], wqt[i * P : (i + 1) * P, :])
        nc.sync.dma_start(wkt_sb[i], wkt[i * P : (i + 1) * P, :])
        nc.sync.dma_start(wvt_sb[i], wvt[i * P : (i + 1) * P, :])
    id32_sb = sb.tile([P, P], F32, name="id32sb", tag="id32sb")
    nc.sync.dma_start(id32_sb, id32)
    id16_sb = sb.tile([P, P], F16, name="id16sb", tag="id16sb")
    nc.sync.dma_start(id16_sb, id16)
    nbkr_sb = sb.tile([1, C], F32, name="nbkrsb", tag="nbkrsb")
    nc.sync.dma_start(nbkr_sb, nbkr)
    bvc_sb = [sb.tile([P, 1], F32, name=f"bvc{i}", tag=f"bvc{i}") for i in range(CH)]
    for i in range(CH):
        nc.sync.dma_start(bvc_sb[i], bvc[i * P : (i + 1) * P, :])

    # ---- G assembly ----
    g_sb = [sb.tile([P, C], F32, name=f"gsb{i}", tag=f"gsb{i}") for i in range(CH)]
    if USE_F32R_G:
        for c in range(CH):
            u = USTART[c]
            nc.vector.tensor_copy(g_sb[c][:, u:], g_ps[c][:, u:])
    else:
        m_sb = [sb.tile([P, C], F32, name=f"msb{i}", tag=f"msb{i}") for i in range(CH)]
        for c in range(CH):
            nc.vector.tensor_copy(m_sb[c], m_ps[c])
            nc.vector.tensor_add(g_sb[c], g_ps[c], m_sb[c])

    # ---- u1 = (Wq S)^T, u2 = (Wk S)^T as [1, C] rows (PE filler work) ----
    u1_ps = ps.tile([1, C], F32, name="u1ps", tag="pb0")
    u2_ps = ps.tile([1, C], F32, name="u2ps", tag="pb1")
    for e in range(CH):
        nc.tensor.matmul(
            u1_ps, lhsT=s_col[e], rhs=wqt_sb[e], start=e == 0, stop=e == CH - 1
        )
    for e in range(CH):
        nc.tensor.matmul(
            u2_ps, lhsT=s_col[e], rhs=wkt_sb[e], start=e == 0, stop=e == CH - 1
        )
    bqr_sb = sb.tile([1, C], F32, name="bqr_sb", tag="bqr_sb")
    nc.sync.dma_start(bqr_sb, bqr)
    bkr_sb = sb.tile([1, C], F32, name="bkr_sb", tag="bkr_sb")
    nc.sync.dma_start(bkr_sb, bkr)
    u1_sb = sb.tile([1, C], F32, name="u1_sb", tag="u1_sb")
    nc.vector.tensor_copy(u1_sb, u1_ps)
    r0_sb = sb.tile([1, C], F32, name="r0_sb", tag="r0_sb")
    nc.vector.tensor_add(r0_sb, u2_ps, nbkr_sb)

    # ---- mirror G lower blocks via PE transpose (f32r mode) ----
    if USE_F32R_G:
        for c in range(CH):
            for d in range(USTART[c] // P):
                tb = ps.tile([P, P], F32, name="tb", tag=f"pb{2 + (c + d) % 2}")
                nc.tensor.transpose(
                    tb, g_sb[d][:, c * P : (c + 1) * P], id32_sb
                )
                nc.vector.tensor_copy(g_sb[c][:, d * P : (d + 1) * P], tb)

    # ---- M^T via PE transposes, then G assembly (split mode only) ----
    if not USE_F32R_G:
        mt_ps = [ps.tile([P, C], F32, name=f"mtps{j}", tag=f"pa{j}") for j in range(CH)]
        for j in range(CH):
            for i in range(CH):
                nc.tensor.transpose(
                    mt_ps[j][:, i * P : (i + 1) * P],
                    m_sb[i][:, j * P : (j + 1) * P],
                    id32_sb,
                )
        for c in range(CH):
            nc.vector.tensor_add(g_sb[c], g_sb[c], mt_ps[c])

    # ---- T1 = G Wk^T (fp32) ----
    t1_ps = [ps.tile([P, C], F32, name=f"t1ps{i}", tag=f"pa{i}") for i in range(CH)]
    for e in range(CH):
        for f in range(CH):
            nc.tensor.matmul(
                t1_ps[e],
                lhsT=g_sb[f][:, e * P : (e + 1) * P],
                rhs=wkt_sb[f],
                start=f == 0,
                stop=f == CH - 1,
            )
    t1_sb = [sb.tile([P, C], F32, name=f"t1sb{i}", tag=f"t1sb{i}") for i in range(CH)]
    for e in range(CH):
        nc.vector.tensor_copy(t1_sb[e], t1_ps[e])

    # ---- logits = Wq T1 + rank-1 bias terms (fp32, accumulated in PSUM) ----
    l_ps = [ps.tile([P, C], F32, name=f"lps{i}", tag=f"pb{i}") for i in range(CH)]
    for c in range(CH):
        for e in range(CH):
            nc.tensor.matmul(
                l_ps[c],
                lhsT=wqt_sb[e][:, c * P : (c + 1) * P],
                rhs=t1_sb[e],
                start=e == 0,
                stop=False,
            )
        nc.tensor.matmul(
            l_ps[c],
            lhsT=bqr_sb[:, c * P : (c + 1) * P],
            rhs=r0_sb,
            start=False,
            stop=False,
        )
        nc.tensor.matmul(
            l_ps[c],
            lhsT=u1_sb[:, c * P : (c + 1) * P],
            rhs=bkr_sb,
            start=False,
            stop=True,
        )

    # ---- v = Wv X + bv (fp16) and out = w v + X, software-pipelined ----
    v_sb = [sb.tile([P, HW], F16, name=f"vsb{i}", tag=f"vsb{i}") for i in range(CH)]

    def v_conv(nt):
        for o in range(CH):
            v_ps = ps.tile([P, 512], F32, name=f"vps{o}", tag=f"pa{o}")
            for c in range(CH):
                nc.tensor.matmul(
                    v_ps,
                    lhsT=wvt_sb[c][:, o * P : (o + 1) * P],
                    rhs=x16_sb[c][:, nt * 512 : (nt + 1) * 512],
                    start=c == 0,
                    stop=c == CH - 1,
                )
            nc.vector.tensor_scalar_add(
                v_sb[o][:, nt * 512 : (nt + 1) * 512], v_ps, bvc_sb[o]
            )

    # v-conv for the first tiles fills PE while softmax runs
    for nt in range(3):
        v_conv(nt)

    # ---- softmax over free dim (rows of L) ----
    w16_sb = [sb.tile([P, C], F16, name=f"w16_{i}", tag=f"w16_{i}") for i in range(CH)]
    for c in range(CH):
        negmx = sb.tile([P, 1], F32, name=f"negmx{c}", tag=f"negmx{c}")
        nc.vector.reduce_max(negmx, l_ps[c], axis=AX, negate=True)
        e_sb = sb.tile([P, C], F32, name="esb", tag="esb", bufs=2)
        ssum = sb.tile([P, 1], F32, name=f"ssum{c}", tag=f"ssum{c}")
        nc.scalar.activation(e_sb, l_ps[c], EXP, bias=negmx, scale=1.0, accum_out=ssum)
        rcp = sb.tile([P, 1], F32, name=f"rcp{c}", tag=f"rcp{c}")
        nc.vector.reciprocal(rcp, ssum)
        nc.vector.tensor_scalar_mul(w16_sb[c], e_sb, rcp)

    # ---- transpose softmax weights (fp16, PE transpose) ----
    wt_sb = [sb.tile([P, C], F16, name=f"wtsb{j}", tag=f"wtsb{j}") for j in range(CH)]
    for j in range(CH):
        wt_ps = ps.tile([P, C], F16, name=f"wtps{j}", tag=f"pb{j}")
        for i in range(CH):
            nc.tensor.transpose(
                wt_ps[:, i * P : (i + 1) * P],
                w16_sb[i][:, j * P : (j + 1) * P],
                id16_sb,
            )
        nc.vector.tensor_copy(wt_sb[j], wt_ps)

    # ---- pipelined: out(nt) interleaved with v_conv(nt+3) ----
    def out_tile(nt):
        for c in range(CH):
            o_ps = ps.tile([P, 512], F32, name=f"ops{c}", tag=f"pb{c}")
            for d in range(CH):
                nc.tensor.matmul(
                    o_ps,
                    lhsT=wt_sb[d][:, c * P : (c + 1) * P],
                    rhs=v_sb[d][:, nt * 512 : (nt + 1) * 512],
                    start=d == 0,
                    stop=d == CH - 1,
                )
            o_sb = so.tile([P, 512], F32, name="osb", tag="osb", bufs=4)
            nc.vector.tensor_add(
                o_sb, o_ps, x16_sb[c][:, nt * 512 : (nt + 1) * 512]
            )
            nc.sync.dma_start(
                out[c * P : (c + 1) * P, nt * 512 : (nt + 1) * 512], o_sb
            )

    for nt in range(NT):
        out_tile(nt)
        if nt + 3 < NT:
            v_conv(nt + 3)

    for pool in (so, st, sb, ps):
        pool.release()


def _build_nc():
    nc = bacc.Bacc(
        "TRN2",
        target_bir_lowering=False,
        debug=False,
        num_devices=B,
        enable_asserts=False,
    )
    io = {}
    dt = nc.dram_tensor
    if USE_F32R_G:
        io["xtr"] = dt("xtr", (HW, C), F32R, kind="ExternalInput").ap()
    else:
        io["xth"] = dt("xth", (HW, C), BF16, kind="ExternalInput").ap()
        io["xtl"] = dt("xtl", (HW, C), BF16, kind="ExternalInput").ap()
    io["x16"] = dt("x16", (C, HW), F16, kind="ExternalInput").ap()
    io["wqt"] = dt("wqt", (C, C), F32, kind="ExternalInput").ap()
    io["wkt"] = dt("wkt", (C, C), F32, kind="ExternalInput").ap()
    io["wvt"] = dt("wvt", (C, C), F16, kind="ExternalInput").ap()
    io["bqr"] = dt("bqr", (1, C), F32, kind="ExternalInput").ap()
    io["bkr"] = dt("bkr", (1, C), F32, kind="ExternalInput").ap()
    io["nbkr"] = dt("nbkr", (1, C), F32, kind="ExternalInput").ap()
    io["bvc"] = dt("bvc", (C, 1), F32, kind="ExternalInput").ap()
    io["id32"] = dt("id32", (P, P), F32, kind="ExternalInput").ap()
    io["id16"] = dt("id16", (P, P), F16, kind="ExternalInput").ap()
    io["out"] = dt("out", (C, HW), F32, kind="ExternalOutput").ap()
    with tile.TileContext(nc) as tc:
        _body(tc, nc, io)
    nc.compile()
    return nc


_NC_CACHE = None


def get_nc():
    global _NC_CACHE
    if _NC_CACHE is None:
        _NC_CACHE = _build_nc()
    return _NC_CACHE


def prep_in_maps(x, wq, bq, wk, bk, wv, bv):
    """Host-side input prep: reshape/transpose/dtype-split only."""
    x = np.asarray(x, dtype=np.float32)
    X = x.reshape(B, C, HW)
    XT = np.ascontiguousarray(X.transpose(0, 2, 1))
    if not USE_F32R_G:
        xth = XT.astype(ml_dtypes.bfloat16)
        xtl = (XT - xth.astype(np.float32)).astype(ml_dtypes.bfloat16)
    x16 = X.astype(np.float16)
    wqt = np.ascontiguousarray(np.asarray(wq, np.float32).T)
    wkt = np.ascontiguousarray(np.asarray(wk, np.float32).T)
    wvt = np.ascontiguousarray(np.asarray(wv, np.float32).T).astype(np.float16)
    bqr = np.asarray(bq, np.float32).reshape(1, C)
    bkr = np.asarray(bk, np.float32).reshape(1, C)
    nbkr = (float(HW) * np.asarray(bk, np.float32)).reshape(1, C)
    bvc = np.asarray(bv, np.float32).reshape(C, 1)
    id32 = np.eye(P, dtype=np.float32)
    id16 = np.eye(P, dtype=np.float16)
    in_maps = []
    for b in range(B):
        m = (
            {"xtr": XT[b]} if USE_F32R_G
            else {"xth": xth[b], "xtl": xtl[b]}
        )
        in_maps.append(
            {
                **m,
                "x16": np.ascontiguousarray(x16[b]),
                "wqt": wqt,
                "wkt": wkt,
                "wvt": wvt,
                "bqr": bqr,
                "bkr": bkr,
                "nbkr": nbkr,
                "bvc": bvc,
                "id32": id32,
                "id16": id16,
            }
        )
    return in_maps


def kernel(x, wq, bq, wk, bk, wv, bv):
    nc = get_nc()
    in_maps = prep_in_maps(x, wq, bq, wk, bk, wv, bv)
    res = run_bass_kernel_spmd(nc, in_maps, core_ids=list(range(B)))
    out = np.stack([res.results[b]["out"] for b in range(B)])
    return out.reshape(B, C, 64, 64).astype(np.float32)


# revision 26
# speedup vs baseline: 1.0213x; 1.0213x over previous
"""Channel-attention (nn_ChannelAttentionModule) Trainium2 kernel.

Math (per batch b):
    X = x[b]  [C, N]  with C=512, N=64*64=4096
    q = Wq X + bq ; k = Wk X + bk ; v = Wv X + bv
    L = q k^T                       [C, C]
    out = softmax(L, -1) v + X      [C, N]

Key restructure: L = Wq G Wk^T + bq (Wk S + N bk)^T + (Wq S) bk^T  (outer
products), where G = X X^T (Gram, symmetric) and S = X 1 (row sums).
G is computed in a single fp16 pass (~11-bit input mantissa, 1 cyc/row
on the PE, fp32 PSUM accumulation) over the upper block-triangle,
mirrored via PE transposes; the two 512^3 projection matmuls run in
true fp32; the v-path runs in fp16.  Softmax logits stay fp32.

Sharding: pure data-parallel, one batch per NeuronCore (B=8, 8 cores).
"""

import numpy as np
import ml_dtypes

import concourse.bass as bass
import concourse.mybir as mybir
import concourse.tile as tile
from concourse import bacc
from concourse.bass_utils import run_bass_kernel_spmd

F32 = mybir.dt.float32
F32R = mybir.dt.float32r
F16 = mybir.dt.float16
AX = mybir.AxisListType.X
EXP = mybir.ActivationFunctionType.Exp

B = 8
C = 512
HW = 64 * 64
P = 128
CH = C // P  # 4 channel chunks
NT = HW // 512  # 8 spatial tiles of 512
NG = 8  # xtr granules (4 spatial chunks each)
# upper-triangle start per G row chunk
USTART = [0, 128, 256, 256]


def _body(tc, nc, io):
    xt16, x16 = io["xt16"], io["x16"]
    wqt, wkt, wvt = io["wqt"], io["wkt"], io["wvt"]
    bqr, bkr, nbkr, bvc = io["bqr"], io["bkr"], io["nbkr"], io["bvc"]
    id32, id16, out = io["id32"], io["id16"], io["out"]

    ps = tc.alloc_tile_pool(name="ps", bufs=1, space="PSUM")
    sb = tc.alloc_tile_pool(name="sb", bufs=1)
    st = tc.alloc_tile_pool(name="st", bufs=3)
    so = tc.alloc_tile_pool(name="so", bufs=2)

    # ---- small consts via HWDGE; bulk weights via SWDGE (gpsimd) ----
    id32_sb = sb.tile([P, P], F32, name="id32sb", tag="id32sb")
    nc.sync.dma_start(id32_sb, id32)
    id16_sb = sb.tile([P, P], F16, name="id16sb", tag="id16sb")
    nc.sync.dma_start(id16_sb, id16)
    bqr_sb = sb.tile([1, C], F32, name="bqr_sb", tag="bqr_sb")
    nc.sync.dma_start(bqr_sb, bqr)
    bkr_sb = sb.tile([1, C], F32, name="bkr_sb", tag="bkr_sb")
    nc.sync.dma_start(bkr_sb, bkr)
    nbkr_sb = sb.tile([1, C], F32, name="nbkrsb", tag="nbkrsb")
    nc.sync.dma_start(nbkr_sb, nbkr)

    wv_sb = sb.tile([P, CH * C], F16, name="wv_sb", tag="wv_sb")
    nc.gpsimd.dma_start(
        wv_sb.rearrange("p (e c) -> p e c", e=CH),
        wvt.rearrange("(e p) c -> p e c", p=P),
    )
    bv_sb = sb.tile([P, CH], F32, name="bv_sb", tag="bv_sb")
    nc.gpsimd.dma_start(
        bv_sb.rearrange("p (e o) -> p e o", e=CH),
        bvc.rearrange("(e p) o -> p e o", p=P),
    )

    x16_sb = [sb.tile([P, HW], F16, name=f"x16_{i}", tag=f"x16_{i}") for i in range(CH)]
    v_sb = [sb.tile([P, HW], F16, name=f"vsb{i}", tag=f"vsb{i}") for i in range(CH)]
    wq_sb = sb.tile([P, CH * C], F32, name="wq_sb", tag="wq_sb")
    wk_sb = sb.tile([P, CH * C], F32, name="wk_sb", tag="wk_sb")

    def wqt_s(e, lo, hi):  # wqt chunk e, free cols [lo:hi]
        return wq_sb[:, e * C + lo : e * C + hi]

    def wkt_s(e, lo, hi):
        return wk_sb[:, e * C + lo : e * C + hi]

    def wvt_s(e, lo, hi):
        return wv_sb[:, e * C + lo : e * C + hi]

    def v_conv(nt, tag):
        for o in range(CH):
            v_ps = ps.tile([P, 512], F32, name=f"vps{o}", tag=f"{tag}{o}")
            for c in range(CH):
                nc.tensor.matmul(
                    v_ps,
                    lhsT=wvt_s(c, o * P, (o + 1) * P),
                    rhs=x16_sb[c][:, nt * 512 : (nt + 1) * 512],
                    start=c == 0,
                    stop=c == CH - 1,
                )
            nc.vector.tensor_scalar_add(
                v_sb[o][:, nt * 512 : (nt + 1) * 512], v_ps, bv_sb[:, o : o + 1]
            )

    # ---- interleaved front: x16/xtr stream + v-conv/G rounds ----
    ar_sb = [
        sb.tile([P, 4 * C], F16, name=f"ar{g}", tag=f"ar{g}") for g in range(NG)
    ]
    xtr3 = xt16.rearrange("(g t p) c -> g p t c", p=P, t=4)
    g_ps = [ps.tile([P, C], F32, name=f"gps{i}", tag=f"pa{i}") for i in range(CH)]

    def x16_load(nt2):
        for c in range(CH):
            nc.gpsimd.dma_start(
                x16_sb[c][:, nt2 * 1024 : (nt2 + 1) * 1024],
                x16[c * P : (c + 1) * P, nt2 * 1024 : (nt2 + 1) * 1024],
            )

    def xtr_load(g2):
        nc.sync.dma_start(ar_sb[g2].rearrange("p (t c) -> p t c", t=4), xtr3[g2])

    def g_pass(g2):
        ar4 = ar_sb[g2]
        for t in range(4):
            n = g2 * 4 + t
            first, last = n == 0, n == 4 * NG - 1
            for c in range(CH):
                u = USTART[c]
                nc.tensor.matmul(
                    g_ps[c][:, u:],
                    lhsT=ar4[:, t * C + c * P : t * C + (c + 1) * P],
                    rhs=ar4[:, t * C + u : (t + 1) * C],
                    start=first,
                    stop=last,
                )

    x16_load(0)
    xtr_load(0)
    xtr_load(1)
    x16_load(1)
    v_conv(0, "pb")
    v_conv(1, "pb")
    g_pass(0)
    g_pass(1)
    xtr_load(2)
    xtr_load(3)
    x16_load(2)
    v_conv(2, "pb")
    v_conv(3, "pb")
    g_pass(2)
    g_pass(3)
    xtr_load(4)
    xtr_load(5)
    nc.gpsimd.dma_start(
        wq_sb.rearrange("p (e c) -> p e c", e=CH),
        wqt.rearrange("(e p) c -> p e c", p=P),
    )
    v_conv(4, "pb")
    v_conv(5, "pb")
    g_pass(4)
    g_pass(5)
    xtr_load(6)
    xtr_load(7)
    x16_load(3)
    nc.gpsimd.dma_start(
        wk_sb.rearrange("p (e c) -> p e c", e=CH),
        wkt.rearrange("(e p) c -> p e c", p=P),
    )
    g_pass(6)
    g_pass(7)

    # ---- S row sums (DVE, from fp16 x) ----
    s_col = [sb.tile([P, 1], F32, name=f"s{i}", tag=f"s{i}") for i in range(CH)]
    for i in range(CH):
        nc.vector.reduce_sum(s_col[i], x16_sb[i], axis=AX)

    # ---- G assembly: copy upper, mirror lower via PE transpose ----
    g_sb = [sb.tile([P, C], F32, name=f"gsb{i}", tag=f"gsb{i}") for i in range(CH)]
    for c in range(CH):
        u = USTART[c]
        nc.vector.tensor_copy(g_sb[c][:, u:], g_ps[c][:, u:])

    # ---- u1 = (Wq S)^T, u2 = (Wk S)^T as [1, C] rows (PE filler) ----
    u1_ps = ps.tile([1, C], F32, name="u1ps", tag="pb0")
    u2_ps = ps.tile([1, C], F32, name="u2ps", tag="pb1")
    for e in range(CH):
        nc.tensor.matmul(
            u1_ps, lhsT=s_col[e], rhs=wqt_s(e, 0, C), start=e == 0, stop=e == CH - 1
        )
    for e in range(CH):
        nc.tensor.matmul(
            u2_ps, lhsT=s_col[e], rhs=wkt_s(e, 0, C), start=e == 0, stop=e == CH - 1
        )
    u1_sb = sb.tile([1, C], F32, name="u1_sb", tag="u1_sb")
    nc.vector.tensor_copy(u1_sb, u1_ps)
    r0_sb = sb.tile([1, C], F32, name="r0_sb", tag="r0_sb")
    nc.vector.tensor_add(r0_sb, u2_ps, nbkr_sb)

    # ---- mirror G lower blocks ----
    for c in range(CH):
        for d in range(USTART[c] // P):
            tb = ps.tile([P, P], F32, name="tb", tag=f"pb{2 + (c + d) % 2}")
            nc.tensor.transpose(tb, g_sb[d][:, c * P : (c + 1) * P], id32_sb)
            nc.vector.tensor_copy(g_sb[c][:, d * P : (d + 1) * P], tb)

    # ---- T1 = G Wk^T (fp32) ----
    t1_ps = [ps.tile([P, C], F32, name=f"t1ps{i}", tag=f"pa{i}") for i in range(CH)]
    for e in range(CH):
        for f in range(CH):
            nc.tensor.matmul(
                t1_ps[e],
                lhsT=g_sb[f][:, e * P : (e + 1) * P],
                rhs=wkt_s(f, 0, C),
                start=f == 0,
                stop=f == CH - 1,
            )
    t1_sb = [sb.tile([P, C], F32, name=f"t1sb{i}", tag=f"t1sb{i}") for i in range(CH)]
    for e in range(CH):
        nc.vector.tensor_copy(t1_sb[e], t1_ps[e])

    # ---- logits = Wq T1 + rank-1 bias terms (fp32, PSUM-accumulated) ----
    l_ps = [ps.tile([P, C], F32, name=f"lps{i}", tag=f"pb{i}") for i in range(CH)]
    for c in range(CH):
        for e in range(CH):
            nc.tensor.matmul(
                l_ps[c],
                lhsT=wqt_s(e, c * P, (c + 1) * P),
                rhs=t1_sb[e],
                start=e == 0,
                stop=False,
            )
        nc.tensor.matmul(
            l_ps[c],
            lhsT=bqr_sb[:, c * P : (c + 1) * P],
            rhs=r0_sb,
            start=False,
            stop=False,
        )
        nc.tensor.matmul(
            l_ps[c],
            lhsT=u1_sb[:, c * P : (c + 1) * P],
            rhs=bkr_sb,
            start=False,
            stop=True,
        )

    # ---- v tile 6 fills PE while softmax runs ----
    v_conv(6, "pa")

    # ---- softmax over rows of L ----
    w16_sb = [sb.tile([P, C], F16, name=f"w16_{i}", tag=f"w16_{i}") for i in range(CH)]
    for c in range(CH):
        negmx = sb.tile([P, 1], F32, name=f"negmx{c}", tag=f"negmx{c}")
        nc.vector.reduce_max(negmx, l_ps[c], axis=AX, negate=True)
        e_sb = sb.tile([P, C], F32, name="esb", tag="esb", bufs=2)
        ssum = sb.tile([P, 1], F32, name=f"ssum{c}", tag=f"ssum{c}")
        nc.scalar.activation(e_sb, l_ps[c], EXP, bias=negmx, scale=1.0, accum_out=ssum)
        rcp = sb.tile([P, 1], F32, name=f"rcp{c}", tag=f"rcp{c}")
        nc.vector.reciprocal(rcp, ssum)
        nc.vector.tensor_scalar_mul(w16_sb[c], e_sb, rcp)

    # ---- transpose softmax weights (fp16, PE) ----
    wt_sb = [sb.tile([P, C], F16, name=f"wtsb{j}", tag=f"wtsb{j}") for j in range(CH)]
    for j in range(CH):
        wt_ps = ps.tile([P, C], F16, name=f"wtps{j}", tag=f"pb{j}")
        for i in range(CH):
            nc.tensor.transpose(
                wt_ps[:, i * P : (i + 1) * P],
                w16_sb[i][:, j * P : (j + 1) * P],
                id16_sb,
            )
        nc.vector.tensor_copy(wt_sb[j], wt_ps)

    # ---- out = w v + x (fp16 matmuls, residual from fp16 x) ----
    def out_tile(nt):
        for c in range(CH):
            o_ps = ps.tile([P, 512], F32, name=f"ops{c}", tag=f"pb{c}")
            for d in range(CH):
                nc.tensor.matmul(
                    o_ps,
                    lhsT=wt_sb[d][:, c * P : (c + 1) * P],
                    rhs=v_sb[d][:, nt * 512 : (nt + 1) * 512],
                    start=d == 0,
                    stop=d == CH - 1,
                )
            o_sb = so.tile([P, 512], F32, name="osb", tag="osb", bufs=4)
            nc.vector.tensor_add(
                o_sb, o_ps, x16_sb[c][:, nt * 512 : (nt + 1) * 512]
            )
            nc.sync.dma_start(
                out[c * P : (c + 1) * P, nt * 512 : (nt + 1) * 512], o_sb
            )

    out_tile(0)
    v_conv(7, "pa")
    for nt in range(1, NT):
        out_tile(nt)

    for pool in (so, st, sb, ps):
        pool.release()


def _build_nc():
    nc = bacc.Bacc(
        "TRN2",
        target_bir_lowering=False,
        debug=False,
        num_devices=B,
        enable_asserts=False,
    )
    io = {}
    dt = nc.dram_tensor
    io["xt16"] = dt("xt16", (HW, C), F16, kind="ExternalInput").ap()
    io["x16"] = dt("x16", (C, HW), F16, kind="ExternalInput").ap()
    io["wqt"] = dt("wqt", (C, C), F32, kind="ExternalInput").ap()
    io["wkt"] = dt("wkt", (C, C), F32, kind="ExternalInput").ap()
    io["wvt"] = dt("wvt", (C, C), F16, kind="ExternalInput").ap()
    io["bqr"] = dt("bqr", (1, C), F32, kind="ExternalInput").ap()
    io["bkr"] = dt("bkr", (1, C), F32, kind="ExternalInput").ap()
    io["nbkr"] = dt("nbkr", (1, C), F32, kind="ExternalInput").ap()
    io["bvc"] = dt("bvc", (C, 1), F32, kind="ExternalInput").ap()
    io["id32"] = dt("id32", (P, P), F32, kind="ExternalInput").ap()
    io["id16"] = dt("id16", (P, P), F16, kind="ExternalInput").ap()
    io["out"] = dt("out", (C, HW), F32, kind="ExternalOutput").ap()
    with tile.TileContext(nc) as tc:
        _body(tc, nc, io)
    nc.compile()
    return nc


_NC_CACHE = None


def get_nc():
    global _NC_CACHE
    if _NC_CACHE is None:
        _NC_CACHE = _build_nc()
    return _NC_CACHE


def prep_in_maps(x, wq, bq, wk, bk, wv, bv):
    """Host-side input prep: reshape/transpose/dtype casts only."""
    x = np.asarray(x, dtype=np.float32)
    X = x.reshape(B, C, HW)
    XT = np.ascontiguousarray(X.transpose(0, 2, 1))
    xt16 = XT.astype(np.float16)
    x16 = X.astype(np.float16)
    wqt = np.ascontiguousarray(np.asarray(wq, np.float32).T)
    wkt = np.ascontiguousarray(np.asarray(wk, np.float32).T)
    wvt = np.ascontiguousarray(np.asarray(wv, np.float32).T).astype(np.float16)
    bqr = np.asarray(bq, np.float32).reshape(1, C)
    bkr = np.asarray(bk, np.float32).reshape(1, C)
    nbkr = (float(HW) * np.asarray(bk, np.float32)).reshape(1, C)
    bvc = np.asarray(bv, np.float32).reshape(C, 1)
    id32 = np.eye(P, dtype=np.float32)
    id16 = np.eye(P, dtype=np.float16)
    in_maps = []
    for b in range(B):
        in_maps.append(
            {
                "xt16": xt16[b],
                "x16": np.ascontiguousarray(x16[b]),
                "wqt": wqt,
                "wkt": wkt,
                "wvt": wvt,
                "bqr": bqr,
                "bkr": bkr,
                "nbkr": nbkr,
                "bvc": bvc,
                "id32": id32,
                "id16": id16,
            }
        )
    return in_maps


def kernel(x, wq, bq, wk, bk, wv, bv):
    nc = get_nc()
    in_maps = prep_in_maps(x, wq, bq, wk, bk, wv, bv)
    res = run_bass_kernel_spmd(nc, in_maps, core_ids=list(range(B)))
    out = np.stack([res.results[b]["out"] for b in range(B)])
    return out.reshape(B, C, 64, 64).astype(np.float32)


# revision 28
# speedup vs baseline: 1.0285x; 1.0070x over previous
"""Channel-attention (nn_ChannelAttentionModule) Trainium2 kernel.

Math (per batch b):
    X = x[b]  [C, N]  with C=512, N=64*64=4096
    q = Wq X + bq ; k = Wk X + bk ; v = Wv X + bv
    L = q k^T                       [C, C]
    out = softmax(L, -1) v + X      [C, N]

Key restructure: L = Wq G Wk^T + bq (Wk S + N bk)^T + (Wq S) bk^T  (outer
products), where G = X X^T (Gram, symmetric) and S = X 1 (row sums).
G is computed in a single fp16 pass (~11-bit input mantissa, 1 cyc/row
on the PE, fp32 PSUM accumulation) over the upper block-triangle,
mirrored via PE transposes; the two 512^3 projection matmuls run in
true fp32; the v-path runs in fp16.  Softmax logits stay fp32.

Sharding: pure data-parallel, one batch per NeuronCore (B=8, 8 cores).
"""

import numpy as np
import ml_dtypes

import concourse.bass as bass
import concourse.mybir as mybir
import concourse.tile as tile
from concourse import bacc
from concourse.bass_utils import run_bass_kernel_spmd

F32 = mybir.dt.float32
F32R = mybir.dt.float32r
F16 = mybir.dt.float16
AX = mybir.AxisListType.X
EXP = mybir.ActivationFunctionType.Exp

B = 8
C = 512
HW = 64 * 64
P = 128
CH = C // P  # 4 channel chunks
NT = HW // 512  # 8 spatial tiles of 512
NG = 8  # xtr granules (4 spatial chunks each)
# upper-triangle start per G row chunk
USTART = [0, 128, 256, 256]


def _body(tc, nc, io):
    xt16, x16 = io["xt16"], io["x16"]
    wqt, wkt, wvt = io["wqt"], io["wkt"], io["wvt"]
    bqr, bkr, nbkr, bvc = io["bqr"], io["bkr"], io["nbkr"], io["bvc"]
    id32, id16, out = io["id32"], io["id16"], io["out"]

    ps = tc.alloc_tile_pool(name="ps", bufs=1, space="PSUM")
    sb = tc.alloc_tile_pool(name="sb", bufs=1)
    st = tc.alloc_tile_pool(name="st", bufs=3)
    so = tc.alloc_tile_pool(name="so", bufs=2)

    wv_sb = sb.tile([P, CH * C], F16, name="wv_sb", tag="wv_sb")
    bv_sb = sb.tile([P, CH], F32, name="bv_sb", tag="bv_sb")
    x16_sb = [sb.tile([P, HW], F16, name=f"x16_{i}", tag=f"x16_{i}") for i in range(CH)]
    v_sb = [sb.tile([P, HW], F16, name=f"vsb{i}", tag=f"vsb{i}") for i in range(CH)]
    wq_sb = sb.tile([P, CH * C], F32, name="wq_sb", tag="wq_sb")
    wk_sb = sb.tile([P, CH * C], F32, name="wk_sb", tag="wk_sb")

    def wqt_s(e, lo, hi):  # wqt chunk e, free cols [lo:hi]
        return wq_sb[:, e * C + lo : e * C + hi]

    def wkt_s(e, lo, hi):
        return wk_sb[:, e * C + lo : e * C + hi]

    def wvt_s(e, lo, hi):
        return wv_sb[:, e * C + lo : e * C + hi]

    def v_conv(nt, tag):
        for o in range(CH):
            v_ps = ps.tile([P, 512], F32, name=f"vps{o}", tag=f"{tag}{o}")
            for c in range(CH):
                nc.tensor.matmul(
                    v_ps,
                    lhsT=wvt_s(c, o * P, (o + 1) * P),
                    rhs=x16_sb[c][:, nt * 512 : (nt + 1) * 512],
                    start=c == 0,
                    stop=c == CH - 1,
                )
            nc.vector.tensor_scalar_add(
                v_sb[o][:, nt * 512 : (nt + 1) * 512], v_ps, bv_sb[:, o : o + 1]
            )

    # ---- interleaved front: x16/xtr stream + v-conv/G rounds ----
    ar_sb = [
        sb.tile([P, 4 * C], F16, name=f"ar{g}", tag=f"ar{g}") for g in range(NG)
    ]
    xtr3 = xt16.rearrange("(g t p) c -> g p t c", p=P, t=4)
    g_ps = [ps.tile([P, C], F32, name=f"gps{i}", tag=f"pa{i}") for i in range(CH)]

    def x16_load(nt2):
        for c in range(CH):
            nc.gpsimd.dma_start(
                x16_sb[c][:, nt2 * 1024 : (nt2 + 1) * 1024],
                x16[c * P : (c + 1) * P, nt2 * 1024 : (nt2 + 1) * 1024],
            )

    def xtr_load(g2):
        nc.sync.dma_start(ar_sb[g2].rearrange("p (t c) -> p t c", t=4), xtr3[g2])

    def g_pass(g2):
        ar4 = ar_sb[g2]
        for t in range(4):
            n = g2 * 4 + t
            first, last = n == 0, n == 4 * NG - 1
            for c in range(CH):
                u = USTART[c]
                nc.tensor.matmul(
                    g_ps[c][:, u:],
                    lhsT=ar4[:, t * C + c * P : t * C + (c + 1) * P],
                    rhs=ar4[:, t * C + u : (t + 1) * C],
                    start=first,
                    stop=last,
                )

    xtr_load(0)
    xtr_load(1)
    nc.sync.dma_start(
        wv_sb.rearrange("p (e c) -> p e c", e=CH),
        wvt.rearrange("(e p) c -> p e c", p=P),
    )
    nc.sync.dma_start(
        bv_sb.rearrange("p (e o) -> p e o", e=CH),
        bvc.rearrange("(e p) o -> p e o", p=P),
    )
    x16_load(0)
    g_pass(0)
    g_pass(1)
    xtr_load(2)
    xtr_load(3)
    x16_load(1)
    v_conv(0, "pb")
    v_conv(1, "pb")
    g_pass(2)
    g_pass(3)
    xtr_load(4)
    xtr_load(5)
    x16_load(2)
    v_conv(2, "pb")
    v_conv(3, "pb")
    g_pass(4)
    g_pass(5)
    x16_load(3)
    xtr_load(6)
    xtr_load(7)
    nc.gpsimd.dma_start(
        wq_sb.rearrange("p (e c) -> p e c", e=CH),
        wqt.rearrange("(e p) c -> p e c", p=P),
    )
    nc.gpsimd.dma_start(
        wk_sb.rearrange("p (e c) -> p e c", e=CH),
        wkt.rearrange("(e p) c -> p e c", p=P),
    )
    s_col = [sb.tile([P, 1], F32, name=f"s{i}", tag=f"s{i}") for i in range(CH)]
    for i in range(CH):
        nc.vector.reduce_sum(s_col[i], x16_sb[i], axis=AX)
    v_conv(4, "pb")
    v_conv(5, "pb")
    g_pass(6)
    g_pass(7)

    # ---- consts needed by the mid/late phases ----
    id32_sb = sb.tile([P, P], F32, name="id32sb", tag="id32sb")
    nc.sync.dma_start(id32_sb, id32)
    id16_sb = sb.tile([P, P], F16, name="id16sb", tag="id16sb")
    nc.sync.dma_start(id16_sb, id16)
    bqr_sb = sb.tile([1, C], F32, name="bqr_sb", tag="bqr_sb")
    nc.sync.dma_start(bqr_sb, bqr)
    bkr_sb = sb.tile([1, C], F32, name="bkr_sb", tag="bkr_sb")
    nc.sync.dma_start(bkr_sb, bkr)
    nbkr_sb = sb.tile([1, C], F32, name="nbkrsb", tag="nbkrsb")
    nc.sync.dma_start(nbkr_sb, nbkr)

    # ---- u1 = (Wq S)^T, u2 = (Wk S)^T (PE filler while DVE copies G) ----
    u1_ps = ps.tile([1, C], F32, name="u1ps", tag="pb0")
    u2_ps = ps.tile([1, C], F32, name="u2ps", tag="pb1")
    for e in range(CH):
        nc.tensor.matmul(
            u1_ps, lhsT=s_col[e], rhs=wqt_s(e, 0, C), start=e == 0, stop=e == CH - 1
        )
    for e in range(CH):
        nc.tensor.matmul(
            u2_ps, lhsT=s_col[e], rhs=wkt_s(e, 0, C), start=e == 0, stop=e == CH - 1
        )

    # ---- G assembly: copy upper, mirror lower via PE transpose ----
    g_sb = [sb.tile([P, C], F32, name=f"gsb{i}", tag=f"gsb{i}") for i in range(CH)]
    for c in range(CH):
        u = USTART[c]
        nc.vector.tensor_copy(g_sb[c][:, u:], g_ps[c][:, u:])
    u1_sb = sb.tile([1, C], F32, name="u1_sb", tag="u1_sb")
    nc.vector.tensor_copy(u1_sb, u1_ps)
    r0_sb = sb.tile([1, C], F32, name="r0_sb", tag="r0_sb")
    nc.vector.tensor_add(r0_sb, u2_ps, nbkr_sb)

    # ---- mirror G lower blocks ----
    for c in range(CH):
        for d in range(USTART[c] // P):
            tb = ps.tile([P, P], F32, name="tb", tag=f"pb{2 + (c + d) % 2}")
            nc.tensor.transpose(tb, g_sb[d][:, c * P : (c + 1) * P], id32_sb)
            nc.vector.tensor_copy(g_sb[c][:, d * P : (d + 1) * P], tb)

    # ---- T1 = G Wk^T (fp32) ----
    t1_ps = [ps.tile([P, C], F32, name=f"t1ps{i}", tag=f"pa{i}") for i in range(CH)]
    for e in range(CH):
        for f in range(CH):
            nc.tensor.matmul(
                t1_ps[e],
                lhsT=g_sb[f][:, e * P : (e + 1) * P],
                rhs=wkt_s(f, 0, C),
                start=f == 0,
                stop=f == CH - 1,
            )
    t1_sb = [sb.tile([P, C], F32, name=f"t1sb{i}", tag=f"t1sb{i}") for i in range(CH)]
    for e in range(CH):
        nc.vector.tensor_copy(t1_sb[e], t1_ps[e])

    # ---- logits = Wq T1 + rank-1 bias terms (fp32, PSUM-accumulated) ----
    l_ps = [ps.tile([P, C], F32, name=f"lps{i}", tag=f"pb{i}") for i in range(CH)]
    for c in range(CH):
        for e in range(CH):
            nc.tensor.matmul(
                l_ps[c],
                lhsT=wqt_s(e, c * P, (c + 1) * P),
                rhs=t1_sb[e],
                start=e == 0,
                stop=False,
            )
        nc.tensor.matmul(
            l_ps[c],
            lhsT=bqr_sb[:, c * P : (c + 1) * P],
            rhs=r0_sb,
            start=False,
            stop=False,
        )
        nc.tensor.matmul(
            l_ps[c],
            lhsT=u1_sb[:, c * P : (c + 1) * P],
            rhs=bkr_sb,
            start=False,
            stop=True,
        )

    # ---- v tile 6 fills PE while softmax runs ----
    v_conv(6, "pa")

    # ---- softmax over rows of L ----
    w16_sb = [sb.tile([P, C], F16, name=f"w16_{i}", tag=f"w16_{i}") for i in range(CH)]
    for c in range(CH):
        negmx = sb.tile([P, 1], F32, name=f"negmx{c}", tag=f"negmx{c}")
        nc.vector.reduce_max(negmx, l_ps[c], axis=AX, negate=True)
        e_sb = sb.tile([P, C], F32, name="esb", tag="esb", bufs=2)
        ssum = sb.tile([P, 1], F32, name=f"ssum{c}", tag=f"ssum{c}")
        nc.scalar.activation(e_sb, l_ps[c], EXP, bias=negmx, scale=1.0, accum_out=ssum)
        rcp = sb.tile([P, 1], F32, name=f"rcp{c}", tag=f"rcp{c}")
        nc.vector.reciprocal(rcp, ssum)
        nc.vector.tensor_scalar_mul(w16_sb[c], e_sb, rcp)

    # ---- transpose softmax weights (fp16, PE) ----
    wt_sb = [sb.tile([P, C], F16, name=f"wtsb{j}", tag=f"wtsb{j}") for j in range(CH)]
    for j in range(CH):
        wt_ps = ps.tile([P, C], F16, name=f"wtps{j}", tag=f"pb{j}")
        for i in range(CH):
            nc.tensor.transpose(
                wt_ps[:, i * P : (i + 1) * P],
                w16_sb[i][:, j * P : (j + 1) * P],
                id16_sb,
            )
        nc.vector.tensor_copy(wt_sb[j], wt_ps)

    # ---- out = w v + x (fp16 matmuls, residual from fp16 x) ----
    def out_tile(nt):
        for c in range(CH):
            o_ps = ps.tile([P, 512], F32, name=f"ops{c}", tag=f"pb{c}")
            for d in range(CH):
                nc.tensor.matmul(
                    o_ps,
                    lhsT=wt_sb[d][:, c * P : (c + 1) * P],
                    rhs=v_sb[d][:, nt * 512 : (nt + 1) * 512],
                    start=d == 0,
                    stop=d == CH - 1,
                )
            o_sb = so.tile([P, 512], F32, name="osb", tag="osb", bufs=4)
            nc.vector.tensor_add(
                o_sb, o_ps, x16_sb[c][:, nt * 512 : (nt + 1) * 512]
            )
            nc.sync.dma_start(
                out[c * P : (c + 1) * P, nt * 512 : (nt + 1) * 512], o_sb
            )

    out_tile(0)
    v_conv(7, "pa")
    for nt in range(1, NT):
        out_tile(nt)

    for pool in (so, st, sb, ps):
        pool.release()


def _build_nc():
    nc = bacc.Bacc(
        "TRN2",
        target_bir_lowering=False,
        debug=False,
        num_devices=B,
        enable_asserts=False,
    )
    io = {}
    dt = nc.dram_tensor
    io["xt16"] = dt("xt16", (HW, C), F16, kind="ExternalInput").ap()
    io["x16"] = dt("x16", (C, HW), F16, kind="ExternalInput").ap()
    io["wqt"] = dt("wqt", (C, C), F32, kind="ExternalInput").ap()
    io["wkt"] = dt("wkt", (C, C), F32, kind="ExternalInput").ap()
    io["wvt"] = dt("wvt", (C, C), F16, kind="ExternalInput").ap()
    io["bqr"] = dt("bqr", (1, C), F32, kind="ExternalInput").ap()
    io["bkr"] = dt("bkr", (1, C), F32, kind="ExternalInput").ap()
    io["nbkr"] = dt("nbkr", (1, C), F32, kind="ExternalInput").ap()
    io["bvc"] = dt("bvc", (C, 1), F32, kind="ExternalInput").ap()
    io["id32"] = dt("id32", (P, P), F32, kind="ExternalInput").ap()
    io["id16"] = dt("id16", (P, P), F16, kind="ExternalInput").ap()
    io["out"] = dt("out", (C, HW), F32, kind="ExternalOutput").ap()
    with tile.TileContext(nc) as tc:
        _body(tc, nc, io)
    nc.compile()
    return nc


_NC_CACHE = None


def get_nc():
    global _NC_CACHE
    if _NC_CACHE is None:
        _NC_CACHE = _build_nc()
    return _NC_CACHE


def prep_in_maps(x, wq, bq, wk, bk, wv, bv):
    """Host-side input prep: reshape/transpose/dtype casts only."""
    x = np.asarray(x, dtype=np.float32)
    X = x.reshape(B, C, HW)
    XT = np.ascontiguousarray(X.transpose(0, 2, 1))
    xt16 = XT.astype(np.float16)
    x16 = X.astype(np.float16)
    wqt = np.ascontiguousarray(np.asarray(wq, np.float32).T)
    wkt = np.ascontiguousarray(np.asarray(wk, np.float32).T)
    wvt = np.ascontiguousarray(np.asarray(wv, np.float32).T).astype(np.float16)
    bqr = np.asarray(bq, np.float32).reshape(1, C)
    bkr = np.asarray(bk, np.float32).reshape(1, C)
    nbkr = (float(HW) * np.asarray(bk, np.float32)).reshape(1, C)
    bvc = np.asarray(bv, np.float32).reshape(C, 1)
    id32 = np.eye(P, dtype=np.float32)
    id16 = np.eye(P, dtype=np.float16)
    in_maps = []
    for b in range(B):
        in_maps.append(
            {
                "xt16": xt16[b],
                "x16": np.ascontiguousarray(x16[b]),
                "wqt": wqt,
                "wkt": wkt,
                "wvt": wvt,
                "bqr": bqr,
                "bkr": bkr,
                "nbkr": nbkr,
                "bvc": bvc,
                "id32": id32,
                "id16": id16,
            }
        )
    return in_maps


def kernel(x, wq, bq, wk, bk, wv, bv):
    nc = get_nc()
    in_maps = prep_in_maps(x, wq, bq, wk, bk, wv, bv)
    res = run_bass_kernel_spmd(nc, in_maps, core_ids=list(range(B)))
    out = np.stack([res.results[b]["out"] for b in range(B)])
    return out.reshape(B, C, 64, 64).astype(np.float32)


# revision 30
# speedup vs baseline: 2.0670x; 2.0097x over previous
"""Channel-attention (nn_ChannelAttentionModule) Trainium2 kernel.

Math (per batch b):
    X = x[b]  [C, N]  with C=512, N=64*64=4096
    q = Wq X + bq ; k = Wk X + bk ; v = Wv X + bv
    L = q k^T                       [C, C]
    out = softmax(L, -1) v + X      [C, N]

Key restructure: L = Wq G Wk^T + bq (Wk S + N bk)^T + (Wq S) bk^T  (outer
products), where G = X X^T (Gram, symmetric) and S = X 1 (row sums).
G is computed in a single fp16 pass (~11-bit input mantissa, 1 cyc/row
on the PE, fp32 PSUM accumulation) over the upper block-triangle,
mirrored via PE transposes; the two 512^3 projection matmuls run in
true fp32; the v-path runs in fp16.  Softmax logits stay fp32.

Sharding: pure data-parallel, one batch per NeuronCore (B=8, 8 cores).
"""

import numpy as np
import ml_dtypes

import concourse.bass as bass
import concourse.mybir as mybir
import concourse.tile as tile
from concourse import bacc
from concourse.bass_utils import run_bass_kernel_spmd

F32 = mybir.dt.float32
F32R = mybir.dt.float32r
F16 = mybir.dt.float16
AX = mybir.AxisListType.X
EXP = mybir.ActivationFunctionType.Exp

B = 8
C = 512
HW = 64 * 64
P = 128
CH = C // P  # 4 channel chunks
NT = HW // 512  # 8 spatial tiles of 512
NG = 8  # xtr granules (4 spatial chunks each)
# upper-triangle start per G row chunk
USTART = [0, 128, 256, 256]


def _body(tc, nc, io):
    xt16, x16 = io["xt16"], io["x16"]
    wqh, wql, wkh, wkl, wvt = io["wqh"], io["wql"], io["wkh"], io["wkl"], io["wvt"]
    bqr, bkr, nbkr, bvc = io["bqr"], io["bkr"], io["nbkr"], io["bvc"]
    id32, id16, out = io["id32"], io["id16"], io["out"]

    ps = tc.alloc_tile_pool(name="ps", bufs=1, space="PSUM")
    sb = tc.alloc_tile_pool(name="sb", bufs=1)
    st = tc.alloc_tile_pool(name="st", bufs=3)
    so = tc.alloc_tile_pool(name="so", bufs=2)

    wv_sb = sb.tile([P, CH * C], F16, name="wv_sb", tag="wv_sb")
    bv_sb = sb.tile([P, CH], F32, name="bv_sb", tag="bv_sb")
    x16_sb = [sb.tile([P, HW], F16, name=f"x16_{i}", tag=f"x16_{i}") for i in range(CH)]
    v_sb = [sb.tile([P, HW], F16, name=f"vsb{i}", tag=f"vsb{i}") for i in range(CH)]
    wqh_sb = sb.tile([P, CH * C], F16, name="wqh_sb", tag="wqh_sb")
    wql_sb = sb.tile([P, CH * C], F16, name="wql_sb", tag="wql_sb")
    wkh_sb = sb.tile([P, CH * C], F16, name="wkh_sb", tag="wkh_sb")
    wkl_sb = sb.tile([P, CH * C], F16, name="wkl_sb", tag="wkl_sb")

    def wslice(tile_, e, lo, hi):
        return tile_[:, e * C + lo : e * C + hi]

    def v_conv(nt, tag):
        for o in range(CH):
            v_ps = ps.tile([P, 512], F32, name=f"vps{o}", tag=f"{tag}{o}")
            for c in range(CH):
                nc.tensor.matmul(
                    v_ps,
                    lhsT=wslice(wv_sb, c, o * P, (o + 1) * P),
                    rhs=x16_sb[c][:, nt * 512 : (nt + 1) * 512],
                    start=c == 0,
                    stop=c == CH - 1,
                )
            nc.vector.tensor_scalar_add(
                v_sb[o][:, nt * 512 : (nt + 1) * 512], v_ps, bv_sb[:, o : o + 1]
            )

    # ---- interleaved front: x16/xtr stream + v-conv/G rounds ----
    ar_sb = [
        sb.tile([P, 4 * C], F16, name=f"ar{g}", tag=f"ar{g}") for g in range(NG)
    ]
    xtr3 = xt16.rearrange("(g t p) c -> g p t c", p=P, t=4)
    g_ps = [ps.tile([P, C], F32, name=f"gps{i}", tag=f"pa{i}") for i in range(CH)]

    def x16_load(nt2):
        for c in range(CH):
            nc.gpsimd.dma_start(
                x16_sb[c][:, nt2 * 1024 : (nt2 + 1) * 1024],
                x16[c * P : (c + 1) * P, nt2 * 1024 : (nt2 + 1) * 1024],
            )

    def xtr_load(g2):
        nc.sync.dma_start(ar_sb[g2].rearrange("p (t c) -> p t c", t=4), xtr3[g2])

    def g_pass(g2):
        ar4 = ar_sb[g2]
        for t in range(4):
            n = g2 * 4 + t
            first, last = n == 0, n == 4 * NG - 1
            for c in range(CH):
                u = USTART[c]
                nc.tensor.matmul(
                    g_ps[c][:, u:],
                    lhsT=ar4[:, t * C + c * P : t * C + (c + 1) * P],
                    rhs=ar4[:, t * C + u : (t + 1) * C],
                    start=first,
                    stop=last,
                )

    xtr_load(0)
    xtr_load(1)
    nc.sync.dma_start(
        wv_sb.rearrange("p (e c) -> p e c", e=CH),
        wvt.rearrange("(e p) c -> p e c", p=P),
    )
    nc.sync.dma_start(
        bv_sb.rearrange("p (e o) -> p e o", e=CH),
        bvc.rearrange("(e p) o -> p e o", p=P),
    )
    x16_load(0)
    g_pass(0)
    g_pass(1)
    xtr_load(2)
    xtr_load(3)
    x16_load(1)
    v_conv(0, "pb")
    v_conv(1, "pb")
    g_pass(2)
    g_pass(3)
    xtr_load(4)
    xtr_load(5)
    x16_load(2)
    v_conv(2, "pb")
    v_conv(3, "pb")
    g_pass(4)
    g_pass(5)
    x16_load(3)
    xtr_load(6)
    xtr_load(7)
    for wtile, wdram in ((wqh_sb, wqh), (wkh_sb, wkh), (wql_sb, wql), (wkl_sb, wkl)):
        nc.gpsimd.dma_start(
            wtile.rearrange("p (e c) -> p e c", e=CH),
            wdram.rearrange("(e p) c -> p e c", p=P),
        )
    s_col = [sb.tile([P, 1], F32, name=f"s{i}", tag=f"s{i}") for i in range(CH)]
    for i in range(CH):
        nc.vector.reduce_sum(s_col[i], x16_sb[i], axis=AX)
    v_conv(4, "pb")
    v_conv(5, "pb")
    g_pass(6)
    g_pass(7)

    # ---- consts needed by the mid/late phases ----
    id32_sb = sb.tile([P, P], F32, name="id32sb", tag="id32sb")
    nc.sync.dma_start(id32_sb, id32)
    id16_sb = sb.tile([P, P], F16, name="id16sb", tag="id16sb")
    nc.sync.dma_start(id16_sb, id16)
    nbkr_sb = sb.tile([1, C], F32, name="nbkrsb", tag="nbkrsb")
    nc.sync.dma_start(nbkr_sb, nbkr)

    # ---- u1 = (Wq S)^T, u2 = (Wk S)^T (fp16-hi; error ~1e-4 on logits) ----
    s16 = [sb.tile([P, 1], F16, name=f"s16_{i}", tag=f"s16_{i}") for i in range(CH)]
    for i in range(CH):
        nc.vector.tensor_copy(s16[i], s_col[i])
    u1_ps = ps.tile([1, C], F32, name="u1ps", tag="pb0")
    u2_ps = ps.tile([1, C], F32, name="u2ps", tag="pb1")
    for e in range(CH):
        nc.tensor.matmul(
            u1_ps, lhsT=s16[e], rhs=wslice(wqh_sb, e, 0, C),
            start=e == 0, stop=e == CH - 1,
        )
    for e in range(CH):
        nc.tensor.matmul(
            u2_ps, lhsT=s16[e], rhs=wslice(wkh_sb, e, 0, C),
            start=e == 0, stop=e == CH - 1,
        )

    # ---- G assembly: copy upper, mirror lower via PE transpose ----
    g_sb = [sb.tile([P, C], F32, name=f"gsb{i}", tag=f"gsb{i}") for i in range(CH)]
    for c in range(CH):
        u = USTART[c]
        nc.vector.tensor_copy(g_sb[c][:, u:], g_ps[c][:, u:])
    u1_sb = sb.tile([1, C], F32, name="u1_sb", tag="u1_sb")
    nc.vector.tensor_copy(u1_sb, u1_ps)
    lhs2 = sb.tile([2, C], F32, name="lhs2", tag="lhs2")
    nc.sync.dma_start(lhs2[0:1, :], bqr)
    nc.sync.dma_start(lhs2[1:2, :], u1_sb)
    rhs2 = sb.tile([2, C], F32, name="rhs2", tag="rhs2")
    nc.vector.tensor_add(rhs2[0:1, :], u2_ps, nbkr_sb)
    nc.sync.dma_start(rhs2[1:2, :], bkr)

    # ---- mirror G lower blocks ----
    for c in range(CH):
        for d in range(USTART[c] // P):
            tb = ps.tile([P, P], F32, name="tb", tag=f"pb{2 + (c + d) % 2}")
            nc.tensor.transpose(tb, g_sb[d][:, c * P : (c + 1) * P], id32_sb)
            nc.vector.tensor_copy(g_sb[c][:, d * P : (d + 1) * P], tb)

    # ---- split G into f16 hi/lo (ACT copies the hi, DVE subs the lo) ----
    gh = [sb.tile([P, C], F16, name=f"gh{i}", tag=f"gh{i}") for i in range(CH)]
    gl = [sb.tile([P, C], F16, name=f"gl{i}", tag=f"gl{i}") for i in range(CH)]
    for c in range(CH):
        nc.scalar.copy(gh[c], g_sb[c])
        nc.vector.tensor_sub(gl[c], g_sb[c], gh[c])

    # ---- T1 = G Wk^T via 3 f16 passes (hi*hi + hi*lo + lo*hi) ----
    t1_ps = [ps.tile([P, C], F32, name=f"t1ps{i}", tag=f"pa{i}") for i in range(CH)]
    for e in range(CH):
        for f in range(CH):
            nc.tensor.matmul(
                t1_ps[e], lhsT=gh[f][:, e * P : (e + 1) * P],
                rhs=wslice(wkh_sb, f, 0, C), start=f == 0, stop=False,
            )
        for f in range(CH):
            nc.tensor.matmul(
                t1_ps[e], lhsT=gh[f][:, e * P : (e + 1) * P],
                rhs=wslice(wkl_sb, f, 0, C), start=False, stop=False,
            )
        for f in range(CH):
            nc.tensor.matmul(
                t1_ps[e], lhsT=gl[f][:, e * P : (e + 1) * P],
                rhs=wslice(wkh_sb, f, 0, C), start=False, stop=f == CH - 1,
            )
    t1h = [sb.tile([P, C], F16, name=f"t1h{i}", tag=f"t1h{i}") for i in range(CH)]
    t1l = [sb.tile([P, C], F16, name=f"t1l{i}", tag=f"t1l{i}") for i in range(CH)]
    for e in range(CH):
        nc.scalar.copy(t1h[e], t1_ps[e])
        nc.vector.tensor_sub(t1l[e], t1_ps[e], t1h[e])

    # ---- logits = Wq T1 + rank-1 bias terms (fp32, PSUM-accumulated) ----
    l_ps = [ps.tile([P, C], F32, name=f"lps{i}", tag=f"pb{i}") for i in range(CH)]
    for c in range(CH):
        for e in range(CH):
            nc.tensor.matmul(
                l_ps[c], lhsT=wslice(wqh_sb, e, c * P, (c + 1) * P),
                rhs=t1h[e], start=e == 0, stop=False,
            )
        for e in range(CH):
            nc.tensor.matmul(
                l_ps[c], lhsT=wslice(wqh_sb, e, c * P, (c + 1) * P),
                rhs=t1l[e], start=False, stop=False,
            )
        for e in range(CH):
            nc.tensor.matmul(
                l_ps[c], lhsT=wslice(wql_sb, e, c * P, (c + 1) * P),
                rhs=t1h[e], start=False, stop=False,
            )
        nc.tensor.matmul(
            l_ps[c], lhsT=lhs2[:, c * P : (c + 1) * P], rhs=rhs2,
            start=False, stop=True,
        )

    # ---- v tile 6 fills PE while softmax runs ----
    v_conv(6, "pa")

    # ---- softmax over rows of L ----
    w16_sb = [sb.tile([P, C], F16, name=f"w16_{i}", tag=f"w16_{i}") for i in range(CH)]
    for c in range(CH):
        negmx = sb.tile([P, 1], F32, name=f"negmx{c}", tag=f"negmx{c}")
        nc.vector.reduce_max(negmx, l_ps[c], axis=AX, negate=True)
        e_sb = sb.tile([P, C], F32, name="esb", tag="esb", bufs=2)
        ssum = sb.tile([P, 1], F32, name=f"ssum{c}", tag=f"ssum{c}")
        nc.scalar.activation(e_sb, l_ps[c], EXP, bias=negmx, scale=1.0, accum_out=ssum)
        rcp = sb.tile([P, 1], F32, name=f"rcp{c}", tag=f"rcp{c}")
        nc.vector.reciprocal(rcp, ssum)
        nc.vector.tensor_scalar_mul(w16_sb[c], e_sb, rcp)

    # ---- transpose softmax weights (fp16, PE) ----
    wt_sb = [sb.tile([P, C], F16, name=f"wtsb{j}", tag=f"wtsb{j}") for j in range(CH)]
    for j in range(CH):
        wt_ps = ps.tile([P, C], F16, name=f"wtps{j}", tag=f"pb{j}")
        for i in range(CH):
            nc.tensor.transpose(
                wt_ps[:, i * P : (i + 1) * P],
                w16_sb[i][:, j * P : (j + 1) * P],
                id16_sb,
            )
        nc.vector.tensor_copy(wt_sb[j], wt_ps)

    # ---- out = w v + x (fp16 matmuls, residual from fp16 x) ----
    def out_tile(nt):
        for c in range(CH):
            o_ps = ps.tile([P, 512], F32, name=f"ops{c}", tag=f"pb{c}")
            for d in range(CH):
                nc.tensor.matmul(
                    o_ps,
                    lhsT=wt_sb[d][:, c * P : (c + 1) * P],
                    rhs=v_sb[d][:, nt * 512 : (nt + 1) * 512],
                    start=d == 0,
                    stop=d == CH - 1,
                )
            o_sb = so.tile([P, 512], F32, name="osb", tag="osb", bufs=4)
            nc.vector.tensor_add(
                o_sb, o_ps, x16_sb[c][:, nt * 512 : (nt + 1) * 512]
            )
            nc.sync.dma_start(
                out[c * P : (c + 1) * P, nt * 512 : (nt + 1) * 512], o_sb
            )

    out_tile(0)
    v_conv(7, "pa")
    for nt in range(1, NT):
        out_tile(nt)

    for pool in (so, st, sb, ps):
        pool.release()


def _build_nc(repeat=1):
    nc = bacc.Bacc(
        "TRN2",
        target_bir_lowering=False,
        debug=False,
        num_devices=B,
        enable_asserts=False,
    )
    io = {}
    dt = nc.dram_tensor
    io["xt16"] = dt("xt16", (HW, C), F16, kind="ExternalInput").ap()
    io["x16"] = dt("x16", (C, HW), F16, kind="ExternalInput").ap()
    io["wqh"] = dt("wqh", (C, C), F16, kind="ExternalInput").ap()
    io["wql"] = dt("wql", (C, C), F16, kind="ExternalInput").ap()
    io["wkh"] = dt("wkh", (C, C), F16, kind="ExternalInput").ap()
    io["wkl"] = dt("wkl", (C, C), F16, kind="ExternalInput").ap()
    io["wvt"] = dt("wvt", (C, C), F16, kind="ExternalInput").ap()
    io["bqr"] = dt("bqr", (1, C), F32, kind="ExternalInput").ap()
    io["bkr"] = dt("bkr", (1, C), F32, kind="ExternalInput").ap()
    io["nbkr"] = dt("nbkr", (1, C), F32, kind="ExternalInput").ap()
    io["bvc"] = dt("bvc", (C, 1), F32, kind="ExternalInput").ap()
    io["id32"] = dt("id32", (P, P), F32, kind="ExternalInput").ap()
    io["id16"] = dt("id16", (P, P), F16, kind="ExternalInput").ap()
    io["out"] = dt("out", (C, HW), F32, kind="ExternalOutput").ap()
    with tile.TileContext(nc) as tc:
        for _ in range(repeat):
            _body(tc, nc, io)
    nc.compile()
    return nc


_NC_CACHE = None


def get_nc():
    global _NC_CACHE
    if _NC_CACHE is None:
        _NC_CACHE = _build_nc()
    return _NC_CACHE


def prep_in_maps(x, wq, bq, wk, bk, wv, bv):
    """Host-side input prep: reshape/transpose/dtype casts only."""
    x = np.asarray(x, dtype=np.float32)
    X = x.reshape(B, C, HW)
    XT = np.ascontiguousarray(X.transpose(0, 2, 1))
    xt16 = XT.astype(np.float16)
    x16 = X.astype(np.float16)
    wqt = np.ascontiguousarray(np.asarray(wq, np.float32).T)
    wkt = np.ascontiguousarray(np.asarray(wk, np.float32).T)
    wqh = wqt.astype(np.float16)
    wql = (wqt - wqh.astype(np.float32)).astype(np.float16)
    wkh = wkt.astype(np.float16)
    wkl = (wkt - wkh.astype(np.float32)).astype(np.float16)
    wvt = np.ascontiguousarray(np.asarray(wv, np.float32).T).astype(np.float16)
    bqr = np.asarray(bq, np.float32).reshape(1, C)
    bkr = np.asarray(bk, np.float32).reshape(1, C)
    nbkr = (float(HW) * np.asarray(bk, np.float32)).reshape(1, C)
    bvc = np.asarray(bv, np.float32).reshape(C, 1)
    id32 = np.eye(P, dtype=np.float32)
    id16 = np.eye(P, dtype=np.float16)
    in_maps = []
    for b in range(B):
        in_maps.append(
            {
                "xt16": xt16[b],
                "x16": np.ascontiguousarray(x16[b]),
                "wqh": wqh,
                "wql": wql,
                "wkh": wkh,
                "wkl": wkl,
                "wvt": wvt,
                "bqr": bqr,
                "bkr": bkr,
                "nbkr": nbkr,
                "bvc": bvc,
                "id32": id32,
                "id16": id16,
            }
        )
    return in_maps


def kernel(x, wq, bq, wk, bk, wv, bv):
    nc = get_nc()
    in_maps = prep_in_maps(x, wq, bq, wk, bk, wv, bv)
    res = run_bass_kernel_spmd(nc, in_maps, core_ids=list(range(B)))
    out = np.stack([res.results[b]["out"] for b in range(B)])
    return out.reshape(B, C, 64, 64).astype(np.float32)
